# revision 33
# baseline (speedup 1.0000x reference)
"""Trainium2 Bass kernel for MineralFusion (dwconv fusion + topk masking + SE).

Self-contained: shards batch across 8 NeuronCores (data parallel), runs a
Bass/Tile kernel per core via run_bass_kernel_spmd, gathers full output.

Design: all depthwise-conv taps run as diagonal-weight matmuls on the
TensorEngine (bf16 operands, fp32 PSUM accumulation) except D_F overflow
taps of the 5x5 on the VectorEngine; VectorEngine also does exact top-30
extraction (max8 + match_replace rounds) and the output assembly. The SE
block runs on TensorE/ScalarE. sigmoid(alpha) and all biases are folded
into weights on the host; the score-conv bias is dropped entirely (per-row
constant shifts cannot change a row's top-k).
"""
import numpy as np
import ml_dtypes

B, C, H, W = 32, 256, 56, 56
K = 30
N_CORES = 8
B_LOC = B // N_CORES          # 4 samples per core
NBLK = C // 128               # 2 channel blocks per sample
NTILES = B_LOC * NBLK         # 8 tiles per core

PW = 64                       # padded row stride (4 + 56 + 4)
NROW = 62                     # 3 + 56 + 3 rows
PLANE = NROW * PW             # 3968
PLANE_X = PLANE + 8           # slack so nominal AP spans stay in range
ORIG = 3 * PW + 4             # interior origin (row 3, col 4)
CMP = 3584                    # compact: 7 chunks x 512 (448 data + 64 junk)
NEG_BIG = -(2.0 ** 100)
POS_BIG = +(2.0 ** 100)

TAPS5 = [(dy, dx) for dy in range(-2, 3) for dx in range(-2, 3)]
TAPS3 = [(dy, dx) for dy in range(-1, 2) for dx in range(-1, 2)]
TAPS7 = [(dy, dx) for dy in range(-3, 4) for dx in range(-3, 4)]

D_F = 7                       # 7x7 taps on DVE (the dy=+3 row); 42 on PE
TAPS7_DVE = [(3, dx) for dx in range(-3, 4)]
PAIRS7 = [((dy, dx), (dy + 1, dx)) for dx in range(-3, 4)
          for dy in (-3, -1, 1)]

LAST = {}


def _pad_view(ap_flat, dy, dx):
    """Interior view [128, 7, 8, 56] of a padded [128, >=PLANE] tile,
    shifted by tap (dy, dx). chunk step = 8*PW = 512."""
    off = ORIG + dy * PW + dx
    v = ap_flat[:, off:off + 7 * 8 * PW]
    return v.rearrange("p (k r w) -> p k r w", k=7, r=8, w=PW)[:, :, :, :56]


def _cmp_view(ap_flat):
    """Data view [128, 7, 8, 56] of a compact [*, CMP] tile whose chunks are
    8 rows x 64 cols (56 data + 8 junk) -- matches the padded-plane geometry."""
    v = ap_flat.rearrange("p (k r w) -> p k r w", k=7, r=8, w=64)
    return v[:, :, :, :56]


def build_nc():
    import concourse.bass as bass
    import concourse.mybir as mybir
    from concourse import bacc, tile

    f32 = mybir.dt.float32
    bf16 = mybir.dt.bfloat16
    fp8 = mybir.dt.float8e4
    AF = mybir.ActivationFunctionType
    OP = mybir.AluOpType

    nc = bacc.Bacc("TRN2", target_bir_lowering=False, debug=False)

    x_d = nc.declare_dram_parameter("x", [B_LOC, C, H, W], f32, isOutput=False)
    dgF_d = nc.declare_dram_parameter("dgF", [NBLK, 128, 26 * 128], fp8, isOutput=False)
    dgS_d = nc.declare_dram_parameter("dgS", [NBLK, 128, 9 * 128], fp8, isOutput=False)
    dg3_d = nc.declare_dram_parameter("dg3", [NBLK, 128, 21 * 2 * 128], fp8, isOutput=False)
    wfD_d = nc.declare_dram_parameter("wfD", [NBLK, 128, max(D_F, 1)], f32, isOutput=False)
    bf_d = nc.declare_dram_parameter("bf", [NBLK, 128, 1], f32, isOutput=False)
    b3_d = nc.declare_dram_parameter("b3p", [NBLK, 128, 1], f32, isOutput=False)
    s1_d = nc.declare_dram_parameter("sew1", [NBLK, 128, 16], f32, isOutput=False)
    s2_d = nc.declare_dram_parameter("sew2", [NBLK, 16, 128], f32, isOutput=False)
    out_d = nc.declare_dram_parameter("out", [B_LOC, C, H, W], f32, isOutput=True)


    with tile.TileContext(nc) as tc:
        with (
            tc.tile_pool(name="wpool", bufs=1) as wpool,
            tc.tile_pool(name="xp", bufs=2) as xp_pool,
            tc.tile_pool(name="xf8", bufs=2) as xf8_pool,
            tc.tile_pool(name="fus", bufs=2) as fus_pool,
            tc.tile_pool(name="fus8", bufs=2) as fus8_pool,
            tc.tile_pool(name="fpart", bufs=2) as fp_pool,
            tc.tile_pool(name="scr", bufs=2) as scr_pool,
            tc.tile_pool(name="msk", bufs=2) as msk_pool,
            tc.tile_pool(name="y0", bufs=3) as y0_pool,
            tc.tile_pool(name="small", bufs=12) as sm_pool,
            tc.tile_pool(name="gs", bufs=5) as gs_pool,
            tc.tile_pool(name="gate", bufs=4) as gate_pool,
            tc.tile_pool(name="hsb", bufs=2) as hsb_pool,
            tc.tile_pool(name="pep", bufs=1, space="PSUM") as pep_pool,
            tc.tile_pool(name="sep", bufs=1, space="PSUM") as sep_pool,
        ):
            # ---- preload weights ----
            dgF_sb = wpool.tile([128, NBLK * 26 * 128], fp8)
            dgS_sb = wpool.tile([128, NBLK * 9 * 128], fp8)
            dg3_sb = wpool.tile([128, NBLK * 21 * 2 * 128], fp8)
            wfD_sb = wpool.tile([128, NBLK * max(D_F, 1)], f32)
            bf_sb = wpool.tile([128, NBLK], f32)
            b3_sb = wpool.tile([128, NBLK], f32)
            s1_sb = wpool.tile([128, NBLK * 16], f32)
            s2_sb = wpool.tile([16, NBLK * 128], f32)
            ones_sb = wpool.tile([128, 512], fp8)
            nc.gpsimd.memset(ones_sb[:], 1.0)
            for blk in range(NBLK):
                nc.sync.dma_start(out=dgF_sb[:, blk * 26 * 128:(blk + 1) * 26 * 128], in_=dgF_d[blk])
                nc.sync.dma_start(out=dgS_sb[:, blk * 9 * 128:(blk + 1) * 9 * 128], in_=dgS_d[blk])
                nc.sync.dma_start(out=dg3_sb[:, blk * 21 * 256:(blk + 1) * 21 * 256], in_=dg3_d[blk])
                nc.sync.dma_start(out=wfD_sb[:, blk * max(D_F, 1):(blk + 1) * max(D_F, 1)], in_=wfD_d[blk])
                nc.sync.dma_start(out=bf_sb[:, blk:blk + 1], in_=bf_d[blk])
                nc.sync.dma_start(out=b3_sb[:, blk:blk + 1], in_=b3_d[blk])
                nc.sync.dma_start(out=s1_sb[:, blk * 16:(blk + 1) * 16], in_=s1_d[blk])
                nc.sync.dma_start(out=s2_sb[:, blk * 128:(blk + 1) * 128], in_=s2_d[blk])

            gsums = {}
            y0s = {}

            def emit_se(t, bd):
                hp = sep_pool.tile([16, 1], f32, tag="sep", name=f"hp{t}")
                for b2 in range(NBLK):
                    nc.tensor.matmul(
                        hp[:], s1_sb[:, b2 * 16:(b2 + 1) * 16],
                        gsums[bd * NBLK + b2][:],
                        start=(b2 == 0), stop=(b2 == NBLK - 1))
                hsb = hsb_pool.tile([16, 1], f32, tag="hsb", name=f"hsb{t}")
                nc.scalar.activation(hsb[:], hp[:], AF.Relu)
                for b2 in range(NBLK):
                    glp = sep_pool.tile([128, 1], f32, tag="sep", name=f"glp{t}_{b2}")
                    nc.tensor.matmul(
                        glp[:], s2_sb[:, b2 * 128:(b2 + 1) * 128], hsb[:],
                        start=True, stop=True)
                    gt = gate_pool.tile([128, 1], f32, tag="gate", name=f"gt{t}_{b2}")
                    nc.scalar.activation(gt[:], glp[:], AF.Sigmoid)
                    nc.vector.tensor_scalar_add(gt[:], gt[:], 1.0)
                    t2 = bd * NBLK + b2
                    outf = fp_pool.tile([128, CMP], f32, tag="fpart",
                                        name=f"outf{t}_{b2}")
                    nc.scalar.activation(_cmp_view(outf[:]),
                                         _cmp_view(y0s[t2][:]),
                                         AF.Copy, bias=0.0, scale=gt[:])
                    dst = out_d[bd, b2 * 128:(b2 + 1) * 128] \
                        .rearrange("c h w -> c (h w)") \
                        .rearrange("c (k r w) -> c k r w", k=7, r=8, w=56)
                    nc.sync.dma_start(out=dst, in_=_cmp_view(outf[:]))

            def pe_conv(psum_t, dg_sb, ntaps, taps, blk, rhs_tile, dveshare=0,
                        bias_lhs=None, bias_rhs=None, blkw=None):
                """Accumulate ntaps diag matmuls into psum_t (7 chunks); the
                optional bias is one more diag matmul against a ones-plane."""
                ngrp = ntaps + (1 if bias_lhs is not None else 0)
                for ti in range(ngrp):
                    if ti < ntaps:
                        dy, dx = taps[ti + dveshare]
                        base = (blk * (blkw or ntaps) + ti) * 128
                        lhs = dg_sb[:, base:base + 128]
                        off = ORIG + dy * PW + dx
                    else:
                        lhs = bias_lhs
                    for ch in range(7):
                        if ti < ntaps:
                            rhs = rhs_tile[:, off + ch * 512: off + ch * 512 + 512]
                        else:
                            rhs = bias_rhs
                        dst = psum_t[:, ch * 512:(ch + 1) * 512]
                        nc.tensor.matmul(dst, lhs, rhs,
                                         start=(ti == 0), stop=(ti == ngrp - 1))

            for t in range(NTILES):
                b, blk = divmod(t, NBLK)
                c0 = blk * 128

                xp = xp_pool.tile([128, PLANE_X], f32)
                nc.gpsimd.memset(xp[:, PLANE:PLANE_X], 0.0)
                nc.gpsimd.memset(xp[:, 0:3 * PW], 0.0)
                nc.gpsimd.memset(xp[:, 59 * PW:PLANE], 0.0)
                lcol = xp[:, 3 * PW:59 * PW].rearrange("p (h w) -> p h w", w=PW)
                nc.gpsimd.memset(lcol[:, :, 0:4], 0.0)
                nc.gpsimd.memset(lcol[:, :, 60:64], 0.0)

                x_src = x_d[b, c0:c0 + 128].rearrange("c h w -> c (h w)") \
                    .rearrange("c (k r w) -> c k r w", k=7, r=8, w=56)
                nc.sync.dma_start(out=_pad_view(xp, 0, 0), in_=x_src)

                xf8 = xf8_pool.tile([128, PLANE_X], fp8)
                nc.gpsimd.memset(xf8[:, PLANE:PLANE_X], 0.0)
                nc.scalar.activation(xf8[:, 0:PLANE], xp[:, 0:PLANE], AF.Copy)

                # ---- DVE share of the 7x7: c3part = sum of D_F taps (off the
                # critical path; runs while PE is busy) ----
                fpart = fp_pool.tile([128, CMP], f32, tag="fpart", name=f"fpart{t}")
                dy, dx = TAPS7_DVE[0]
                nc.vector.tensor_scalar(
                    _cmp_view(fpart[:]), _pad_view(xp, dy, dx),
                    wfD_sb[:, blk * D_F:blk * D_F + 1], b3_sb[:, blk:blk + 1],
                    OP.mult, OP.add)
                for i in range(1, D_F):
                    dy, dx = TAPS7_DVE[i]
                    nc.vector.scalar_tensor_tensor(
                        _cmp_view(fpart[:]), _pad_view(xp, dy, dx),
                        wfD_sb[:, blk * D_F + i:blk * D_F + i + 1],
                        _cmp_view(fpart[:]), OP.mult, OP.add)

                # ---- fused' = a*1024*(conv5x5(x, w12) + b12) on PE (fp8) ----
                fus_p = pep_pool.tile([128, CMP], f32, tag="pep", name=f"fusp{t}")
                ngrp5 = 16  # 10 pairs + 5 singles + bias
                gi = 0
                for dx in range(-2, 3):
                    for dy in (-2, 0):
                        base = (blk * 26 + 2 * (gi if True else 0)) * 128
                        base = (blk * 26 + [0, 2, 4, 6, 8, 10, 12, 14, 16, 18][
                            (dx + 2) * 2 + (dy + 2) // 2]) * 128
                        lhs = dgF_sb[:, base:base + 256] \
                            .rearrange("p (i m) -> p i m", i=2, m=128)
                        off0 = ORIG + dy * PW + dx
                        for ch in range(7):
                            rhs = bass.AP(xf8[:].tensor,
                                          xf8[:].offset + off0 + ch * 512,
                                          [[xf8[:].ap[0][0], 128], [PW, 2], [1, 512]])
                            nc.tensor.matmul(fus_p[:, ch * 512:(ch + 1) * 512],
                                             lhs, rhs, start=(gi == 0), stop=False,
                                             perf_mode=mybir.MatmulPerfMode.DoubleRow)
                        gi += 1
                for si, dx in enumerate(range(-2, 3)):  # singles: dy=+2 row
                    base = (blk * 26 + 20 + si) * 128
                    lhs = dgF_sb[:, base:base + 128]
                    off0 = ORIG + 2 * PW + dx
                    for ch in range(7):
                        rhs = xf8[:, off0 + ch * 512: off0 + ch * 512 + 512]
                        nc.tensor.matmul(fus_p[:, ch * 512:(ch + 1) * 512],
                                         lhs, rhs, start=False, stop=False)
                bias_lhs = dgF_sb[:, (blk * 26 + 25) * 128:(blk * 26 + 26) * 128]
                for ch in range(7):
                    nc.tensor.matmul(fus_p[:, ch * 512:(ch + 1) * 512],
                                     bias_lhs, ones_sb[:, 0:512],
                                     start=False, stop=True)
                fus = fus_pool.tile([128, PLANE], bf16)
                nc.gpsimd.memset(fus[:, 0:3 * PW], 0.0)
                nc.gpsimd.memset(fus[:, 59 * PW:PLANE], 0.0)
                fcol = fus[:, 3 * PW:59 * PW].rearrange("p (h w) -> p h w", w=PW)
                nc.gpsimd.memset(fcol[:, :, 0:4], 0.0)
                nc.gpsimd.memset(fcol[:, :, 60:64], 0.0)
                fus8 = fus8_pool.tile([128, PLANE], fp8)
                nc.gpsimd.memset(fus8[:, 0:3 * PW], 0.0)
                nc.gpsimd.memset(fus8[:, 59 * PW:PLANE], 0.0)
                f8col = fus8[:, 3 * PW:59 * PW].rearrange("p (h w) -> p h w", w=PW)
                nc.gpsimd.memset(f8col[:, :, 0:4], 0.0)
                nc.gpsimd.memset(f8col[:, :, 60:64], 0.0)
                nc.scalar.activation(_pad_view(fus8, 0, 0), _cmp_view(fus_p[:]),
                                     AF.Copy, scale=1.0 / 8.0)
                nc.scalar.activation(_pad_view(fus, 0, 0), _cmp_view(fus_p[:]),
                                     AF.Copy, scale=1.0 / 1024.0)

                # ---- scores = conv3x3(fused') on PE, fp8 pairs (scale-free)
                scr_p = pep_pool.tile([128, CMP], f32, tag="pep", name=f"scrp{t}")
                for pi, dx in enumerate(range(-1, 2)):
                    base = (blk * 9 + 2 * pi) * 128
                    lhs = dgS_sb[:, base:base + 256] \
                        .rearrange("p (i m) -> p i m", i=2, m=128)
                    off0 = ORIG - PW + dx
                    for ch in range(7):
                        rhs = bass.AP(fus8[:].tensor,
                                      fus8[:].offset + off0 + ch * 512,
                                      [[fus8[:].ap[0][0], 128], [PW, 2], [1, 512]])
                        nc.tensor.matmul(scr_p[:, ch * 512:(ch + 1) * 512],
                                         lhs, rhs, start=(pi == 0), stop=False,
                                         perf_mode=mybir.MatmulPerfMode.DoubleRow)
                for si, dx in enumerate(range(-1, 2)):
                    base = (blk * 9 + 6 + si) * 128
                    lhs = dgS_sb[:, base:base + 128]
                    off0 = ORIG + PW + dx
                    for ch in range(7):
                        rhs = fus8[:, off0 + ch * 512: off0 + ch * 512 + 512]
                        nc.tensor.matmul(scr_p[:, ch * 512:(ch + 1) * 512],
                                         lhs, rhs, start=False, stop=(si == 2))
                scr = scr_pool.tile([128, CMP], f32)
                scr3 = scr[:].rearrange("p (r w) -> p r w", r=56, w=64)
                nc.gpsimd.memset(scr3[:, :, 56:64], NEG_BIG)
                nc.scalar.activation(_cmp_view(scr[:]), _cmp_view(scr_p[:]), AF.Copy)

                # ---- top-30 extraction ----
                rv = [sm_pool.tile([128, 8], f32, tag="rv", name=f"rv{t}_{r}")
                      for r in range(4)]
                for r in range(3):
                    nc.vector.max(rv[r][:], scr[:])
                    nc.vector.match_replace(scr[:], rv[r][:], scr[:], NEG_BIG)
                nc.vector.max(rv[3][:], scr[:])
                m4 = sm_pool.tile([128, 8], f32, tag="rv", name=f"m4{t}")
                nc.vector.tensor_copy(m4[:, 0:6], rv[3][:, 0:6])
                nc.vector.memset(m4[:, 6:8], POS_BIG)
                nc.vector.match_replace(scr[:], m4[:], scr[:], NEG_BIG)

                # ---- PE: c3'(rest) -> fused -> scores, one psum slot ----
                c3_p = pep_pool.tile([128, CMP], f32, tag="pep", name=f"c3p{t}")
                pstep = xf8[:].ap[0][0]
                for pi, ((dy, dx), _) in enumerate(PAIRS7):
                    base = (blk * 21 + pi) * 256
                    lhs = dg3_sb[:, base:base + 256] \
                        .rearrange("p (i m) -> p i m", i=2, m=128)
                    off0 = ORIG + dy * PW + dx
                    for ch in range(7):
                        # N=512 contiguous (junk cols land in psum junk slots)
                        rhs = bass.AP(xf8[:].tensor,
                                      xf8[:].offset + off0 + ch * 512,
                                      [[pstep, 128], [PW, 2], [1, 512]])
                        dst = c3_p[:, ch * 512:(ch + 1) * 512]
                        nc.tensor.matmul(dst, lhs, rhs, start=(pi == 0),
                                         stop=(pi == 20),
                                         perf_mode=mybir.MatmulPerfMode.DoubleRow)

                # y0 = (x + c3_pe) + c3part  (frees the psum slot fast)
                y0 = y0_pool.tile([128, CMP], bf16)
                nc.vector.scalar_tensor_tensor(
                    _cmp_view(y0[:]), _cmp_view(c3_p[:]), 1.0 / 1024.0,
                    _pad_view(xp, 0, 0), OP.mult, OP.add)
                nc.vector.scalar_tensor_tensor(
                    _cmp_view(y0[:]), _cmp_view(fpart[:]), 1.0,
                    _cmp_view(y0[:]), OP.mult, OP.add)
                y0s[t] = y0

                # mask = (scr == NEG_BIG); o1 = mask * fused' (in place)
                msk = msk_pool.tile([128, CMP], bf16)
                nc.vector.tensor_scalar(msk[:], scr[:], NEG_BIG, None, OP.is_equal)
                nc.vector.tensor_tensor(_cmp_view(msk[:]), _cmp_view(msk[:]),
                                        _pad_view(fus, 0, 0), OP.mult)

                # ---- y = o1 + y0 ; gsum ----
                gs = gs_pool.tile([128, 1], f32)
                nc.vector.scalar_tensor_tensor(
                    _cmp_view(y0[:]), _cmp_view(msk[:]), 1.0, _cmp_view(y0[:]),
                    OP.mult, OP.add, accum_out=gs[:])
                gsums[t] = gs

                # SE for sample bd is emitted one tile later so PE/ACT have
                # conv work queued ahead of the gsum-dependent matmuls.
                if t >= 2 and blk == 0:
                    emit_se(t, (t - 2) // NBLK)
            emit_se(NTILES + 1, B_LOC - 1)

    nc.compile()
    return nc


def _diag_pack(w, scale=1.0):
    """w: [C, T] per-channel tap weights -> [NBLK, 128, T*128] bf16 diagonals."""
    T = w.shape[1]
    d = np.zeros((NBLK, 128, T, 128), dtype=np.float32)
    blk, ch = np.divmod(np.arange(C), 128)
    d[blk[:, None], ch[:, None], np.arange(T)[None, :], ch[:, None]] = w * scale
    return np.ascontiguousarray(
        d.reshape(NBLK, 128, T * 128).astype(ml_dtypes.bfloat16))


def mybir_np_fp8():
    import concourse.mybir as mybir
    return mybir.dt.np(mybir.dt.float8e4)


def _host_prep(inputs):
    x = np.ascontiguousarray(inputs["x"], dtype=np.float32)
    w1 = np.asarray(inputs["w1"], dtype=np.float32)
    b1 = np.asarray(inputs["b1"], dtype=np.float32)
    w2 = np.asarray(inputs["w2"], dtype=np.float32)
    b2 = np.asarray(inputs["b2"], dtype=np.float32)
    w3 = np.asarray(inputs["w3"], dtype=np.float32)
    b3 = np.asarray(inputs["b3"], dtype=np.float32)
    ws = np.asarray(inputs["ws"], dtype=np.float32)
    se_w1 = np.asarray(inputs["se_w1"], dtype=np.float32)
    se_w2 = np.asarray(inputs["se_w2"], dtype=np.float32)
    alpha = float(np.asarray(inputs["alpha"]))

    a = float(1.0 / (1.0 + np.exp(-alpha)))

    # fused' = a * (conv3(x,w1)+b1 + conv5(x,w2)+b2) as one scaled 5x5
    w12 = w2.copy()
    w12[:, :, 1:4, 1:4] += w1
    w12 = (a * w12)[:, 0].reshape(C, 25)
    b12 = a * (b1 + b2)
    w3p = ((1.0 - a) * w3)[:, 0].reshape(C, 49)
    b3p = (1.0 - a) * b3
    wsf = ws[:, 0].reshape(C, 9)

    w3g = w3p.reshape(C, 7, 7)                          # [c, dy+3, dx+3]
    wfD = np.ascontiguousarray(
        w3g[:, 6, :].reshape(NBLK, 128, 7))             # dy=+3 row on DVE
    # dgF fp8 layout: 10 (dy,dy+1)-pairs [(dx,-2),(dx,0) for dx in -2..2],
    # then 5 dy=+2 singles, then the bias diag; all x1024
    w5 = w12.reshape(C, 5, 5)                          # [c, dy+2, dx+2]
    f8m = mybir_np_fp8()
    dF = np.zeros((NBLK, 128, 26, 128), dtype=np.float32)
    blkv, chv = np.divmod(np.arange(C), 128)
    col = 0
    for dx in range(-2, 3):
        for dy in (-2, 0):
            for i in (0, 1):
                dF[blkv, chv, col + i, chv] = w5[:, dy + 2 + i, dx + 2] * 1024.0
            col += 2
    for si, dx in enumerate(range(-2, 3)):
        dF[blkv, chv, 20 + si, chv] = w5[:, 4, dx + 2] * 1024.0
    dF[blkv, chv, 25, chv] = b12 * 1024.0
    dgF = np.ascontiguousarray(dF.reshape(NBLK, 128, 26 * 128).astype(f8m))
    w3s = wsf.reshape(C, 3, 3)                          # [c, dy+1, dx+1]
    dS = np.zeros((NBLK, 128, 9, 128), dtype=np.float32)
    for pi, dx in enumerate(range(-1, 2)):
        for i in (0, 1):
            dS[blkv, chv, 2 * pi + i, chv] = w3s[:, i, dx + 1] * 1024.0
    for si, dx in enumerate(range(-1, 2)):
        dS[blkv, chv, 6 + si, chv] = w3s[:, 2, dx + 1] * 1024.0
    dgS = np.ascontiguousarray(dS.reshape(NBLK, 128, 9 * 128).astype(f8m))
    # fp8 DoubleRow pairs: for dx in -3..3, dy in (-3,-1,1): (dy, dy+1)
    import concourse.mybir as mybir
    f8 = mybir.dt.np(mybir.dt.float8e4)
    d = np.zeros((NBLK, 128, 21, 2, 128), dtype=np.float32)
    blkv, chv = np.divmod(np.arange(C), 128)
    pi = 0
    pairs = [(dy, dx) for dx in range(-3, 4) for dy in (-3, -1, 1)]
    for pi, (dy, dx) in enumerate(pairs):
        for i in (0, 1):
            d[blkv, chv, pi, i, chv] = w3g[:, dy + 3 + i, dx + 3] * 1024.0
    dg3 = np.ascontiguousarray(
        d.reshape(NBLK, 128, 21 * 2 * 128).astype(f8))

    s1 = (se_w1 / float(H * W)).T.reshape(NBLK, 128, 16)
    s2 = se_w2.T.reshape(16, NBLK, 128).transpose(1, 0, 2)

    common = {
        "dgF": dgF, "dgS": dgS, "dg3": dg3,
        "wfD": np.ascontiguousarray(wfD, np.float32),
        "bf": np.ascontiguousarray(b12.reshape(NBLK, 128, 1), np.float32),
        "b3p": np.ascontiguousarray(b3p.reshape(NBLK, 128, 1), np.float32),
        "sew1": np.ascontiguousarray(s1, np.float32),
        "sew2": np.ascontiguousarray(s2, np.float32),
    }
    return x, common


def kernel(**inputs):
    from concourse.bass_utils import run_bass_kernel_spmd

    x, common = _host_prep(inputs)
    nc = build_nc()

    in_maps = []
    for i in range(N_CORES):
        m = {"x": np.ascontiguousarray(x[i * B_LOC:(i + 1) * B_LOC])}
        m.update(common)
        in_maps.append(m)

    res = run_bass_kernel_spmd(nc, in_maps, core_ids=list(range(N_CORES)))
    LAST.clear()
    LAST["exec_time_ns"] = res.exec_time_ns
    LAST["mean_exec_time_ns"] = res.mean_exec_time_ns
    out = np.concatenate([res.results[i]["out"] for i in range(N_CORES)], axis=0)
    return out
